# revision 1
# baseline (speedup 1.0000x reference)
"""Chamfer distance kernel for 8x Trainium2 NeuronCores (Bass/Tile).

Problem: xyz1 [2,8192,3] f32, xyz2 [2,8192,3] f32 ->
  dist1 [2,8192] f32, dist2 [2,8192] f32, idx1 [2,8192] i32, idx2 [2,8192] i32
  (squared L2 nearest-neighbor distances + argmins, both directions).

Sharding: core c owns rows c*1024:(c+1)*1024 of xyz1 (forward direction:
dist1/idx1 shard, min over full xyz2) AND rows c*1024:(c+1)*1024 of xyz2
(reverse direction: dist2/idx2 shard, min over full xyz1). Each core's
outputs are exact output shards -> host just concatenates (no collectives).

Math: d[q,j] = |q|^2 + |db_j|^2 - 2 q.db_j.  The device computes
  e[q,j] = 2 q.db_j - |db_j|^2   (so argmax_j e = argmin_j d),
then dist = |q|^2 - max_j e on the host.  e is produced by ONE bf16 matmul
with K=21 packed rows: each fp32 operand is split into 3 bf16 limbs
(h+m+l), keeping all product terms down to ~2^-27 relative, and -|db|^2 is
folded in via constant-one lhs rows. This gives fp32-grade accuracy at
bf16 matmul speed (1 cycle/row on PE instead of 4 for native fp32).

Per 128-query row-tile the device does:
  - 16 matmuls [128,512] into PSUM (4 groups of 4 banks)
  - PSUM->SBUF copies on the scalar engine
  - value pass: 4 halving elementwise-max folds + reduce_max (DVE)
  - argmin: one scalar_tensor_tensor pass: (e == gmax) * iota, accum_out
    sums the matching index (exact when the row max is unique; rare exact
    ties are detected and fixed up on the host).
"""

import numpy as np
import ml_dtypes

import concourse.bacc as bacc
import concourse.mybir as mybir
import concourse.tile as tile
from concourse.bass_utils import run_bass_kernel_spmd

BF16 = ml_dtypes.bfloat16
F32 = np.float32

NCORES = 8
B, N, M, C = 2, 8192, 8192, 3
SLAB = N // NCORES            # 1024 queries per core per problem
NPROB = 2 * B                 # (fwd,b0),(fwd,b1),(rev,b0),(rev,b1)
K = 21                        # packed contraction rows
TQ = 128                      # queries per row-tile (partitions)
NT = SLAB // TQ               # 8 row-tiles per problem
MCH = 512                     # matmul free chunk (one PSUM bank)
PSW = 2048                    # psum tile width (4 banks)
NG = M // PSW                 # 4 psum groups per row-tile


def _split3(x):
    """fp32 -> three bf16 limbs (as fp32 arrays) with x ~= h+m+l to ~2^-27."""
    x = x.astype(F32)
    h = x.astype(BF16)
    r = (x - h.astype(F32)).astype(F32)
    m = r.astype(BF16)
    r2 = (r - m.astype(F32)).astype(F32)
    l = r2.astype(BF16)
    return h.astype(F32), m.astype(F32), l.astype(F32)


def _pack_terms(q, db, neg_sqdb):
    """Build the K=21 (lhs_col, rhs_row) packed operands.

    q: [Nq,3] fp32 (queries, ALREADY scaled by 2)
    db: [M,3] fp32, neg_sqdb: [M] fp32 (= -|db|^2)
    Returns lhsT [K,Nq] bf16, rhs [K,M] bf16.
    Term order keeps partial sums well-scaled: big terms first, then
    progressively smaller correction terms.
    """
    h1, m1, l1 = _split3(q)
    h2, m2, l2 = _split3(db)
    sh, sm, sl = _split3(neg_sqdb)
    ones = np.ones(q.shape[0], F32)
    lhs_rows, rhs_rows = [], []

    def add(lc, rr):
        lhs_rows.append(lc)
        rhs_rows.append(rr)

    for c in range(3):
        add(h1[:, c], h2[:, c])
    add(ones, sh)
    for c in range(3):
        add(h1[:, c], m2[:, c])
        add(m1[:, c], h2[:, c])
    add(ones, sm)
    for c in range(3):
        add(m1[:, c], m2[:, c])
        add(h1[:, c], l2[:, c])
        add(l1[:, c], h2[:, c])
    add(ones, sl)
    assert len(lhs_rows) == K
    lhsT = np.stack(lhs_rows, 0).astype(BF16)
    rhs = np.stack(rhs_rows, 0).astype(BF16)
    return lhsT, rhs


PROBW = SLAB + M  # per-problem packed operand width (lhs cols ++ rhs cols)


def _build_nc():
    # Bacc (not plain Bass): its compile() pipeline moves matmul waits onto
    # ldweights and splits multi-wait instructions via event semaphores —
    # TRN2 HW allows at most 1 sync wait per instruction.
    nc = bacc.Bacc("TRN2", target_bir_lowering=False, debug=False)
    comb_d = nc.dram_tensor("comb", [K, NPROB * PROBW], mybir.dt.bfloat16,
                            kind="ExternalInput")
    # one packed output: per problem p (72 cols each): cols [t*8, t*8+8) =
    # top-8 values of row-tile t (InstMax writes them directly; col t*8 is
    # the max), col 64+t = argmin index of row-tile t.
    outv_d = nc.dram_tensor("outv", [TQ, NPROB * (8 * NT + NT)],
                            mybir.dt.float32, kind="ExternalOutput")

    with tile.TileContext(nc) as tc:
        with (
            tc.tile_pool(name="const", bufs=1) as constp,
            tc.tile_pool(name="ebuf", bufs=2) as ep,
            tc.tile_pool(name="psum", bufs=2, space="PSUM") as pp,
        ):
            # All matmul operands preloaded into one persistent tile.
            # Split into one DMA per problem: the comb tensor is skinny
            # (21 partitions), so one mega-DMA takes ~40us and stalls the
            # whole pipeline at startup; per-problem DMAs let problem 0
            # start ~4x earlier while the rest stream in behind compute.
            comb_t = constp.tile([K, NPROB * PROBW], mybir.dt.bfloat16)
            # problem 0 split finer still so its first matmuls start asap
            for a, b_ in ((0, SLAB), (SLAB, SLAB + M // 2),
                          (SLAB + M // 2, PROBW)):
                nc.sync.dma_start(comb_t[:, a:b_], comb_d[:, a:b_])
            for p in range(1, NPROB):
                sl_p = slice(p * PROBW, (p + 1) * PROBW)
                nc.sync.dma_start(comb_t[:, sl_p], comb_d[:, sl_p])
            # iota generated on-device (saves a DMA + SBUF)
            iota_t = constp.tile([TQ, M], mybir.dt.uint16)
            nc.gpsimd.iota(iota_t[:], [[1, M]], channel_multiplier=0)
            outv_t = constp.tile([TQ, NPROB * (8 * NT + NT)], mybir.dt.float32)
            for p in range(NPROB):
                ob = p * (8 * NT + NT)
                base = p * PROBW
                for t in range(NT):
                    e_t = ep.tile([TQ, M], mybir.dt.float32, tag="e")
                    for g in range(NG):
                        ps_t = pp.tile([TQ, PSW], mybir.dt.float32, tag="ps")
                        for j in range(PSW // MCH):
                            ch = g * (PSW // MCH) + j
                            o = base + SLAB + ch * MCH
                            nc.tensor.matmul(
                                ps_t[:, j * MCH:(j + 1) * MCH],
                                comb_t[:, base + t * TQ:base + (t + 1) * TQ],
                                comb_t[:, o:o + MCH],
                                start=True, stop=True,
                            )
                        dst = e_t[:, g * PSW:(g + 1) * PSW]
                        nc.scalar.copy(dst, ps_t[:])
                    # value pass: single InstMax (top-8 per partition),
                    # written straight into the output tile; col 0 = max
                    m8 = outv_t[:, ob + t * 8:ob + (t + 1) * 8]
                    nc.vector.max(m8, e_t[:])
                    # argmin: (e == gmax) * iota, accumulated over the row.
                    # In-place over e_t (e is dead afterwards).
                    nc.vector.scalar_tensor_tensor(
                        e_t[:], e_t[:], m8[:, 0:1], iota_t[:],
                        op0=mybir.AluOpType.is_equal,
                        op1=mybir.AluOpType.mult,
                        accum_out=outv_t[:, ob + 8 * NT + t:ob + 8 * NT + t + 1],
                    )
            nc.sync.dma_start(outv_d[:], outv_t[:])
    nc.compile()
    return nc


_NC = None
LAST_RESULTS = None  # most recent BassKernelResults (for profiling harnesses)


def _get_nc():
    global _NC
    if _NC is None:
        _NC = _build_nc()
    return _NC


def _prep_inputs(xyz1, xyz2):
    """Build per-core in_maps. Returns (in_maps, sq1, sq2)."""
    xyz1 = np.asarray(xyz1, F32)
    xyz2 = np.asarray(xyz2, F32)
    sq1 = (xyz1.astype(np.float64) ** 2).sum(-1).astype(F32)  # [B,N]
    sq2 = (xyz2.astype(np.float64) ** 2).sum(-1).astype(F32)  # [B,M]

    combs = [np.empty((K, NPROB * PROBW), BF16) for _ in range(NCORES)]
    for b in range(B):
        for rev in (0, 1):
            p = 2 * rev + b
            qsrc = xyz2[b] if rev else xyz1[b]
            dbsrc = xyz1[b] if rev else xyz2[b]
            nsq = -(sq1[b] if rev else sq2[b])
            lhsT_full, rhs = _pack_terms((2.0 * qsrc).astype(F32), dbsrc, nsq)
            for c in range(NCORES):
                sl = slice(p * PROBW, p * PROBW + SLAB)
                combs[c][:, sl] = lhsT_full[:, c * SLAB:(c + 1) * SLAB]
                combs[c][:, p * PROBW + SLAB:(p + 1) * PROBW] = rhs
    in_maps = [{"comb": combs[c]} for c in range(NCORES)]
    return in_maps, sq1, sq2


def _sim_e_row(q_row, db, neg_sqdb):
    """Recompute one query's e row in numpy with the same 21-term fp32
    accumulation (for rare tie fixup)."""
    lhsT, rhs = _pack_terms((2.0 * q_row[None, :]).astype(F32), db, neg_sqdb)
    acc = np.zeros(db.shape[0], F32)
    for k in range(K):
        acc = (acc + lhsT[k, 0].astype(F32) * rhs[k].astype(F32)).astype(F32)
    return acc


def _fixup(idx, dist, q_pts, db_pts, neg_sq):
    """Detect rows where the argsum produced a bogus index (exact fp32 ties)
    and recompute those rows on the host."""
    n = idx.shape[0]
    idx_i = np.clip(idx.astype(np.int64), 0, M - 1)
    dd = ((q_pts.astype(np.float64) -
           db_pts[idx_i].astype(np.float64)) ** 2).sum(-1)
    bad = (np.abs(dd - dist.astype(np.float64)) > 1e-4) | (idx > M - 1) | (idx < 0)
    out = idx_i.astype(np.int32)
    for r in np.nonzero(bad)[0]:
        e_row = _sim_e_row(q_pts[r], db_pts, neg_sq)
        out[r] = np.int32(e_row.argmax())
    return out


def kernel(xyz1, xyz2):
    xyz1 = np.asarray(xyz1, F32)
    xyz2 = np.asarray(xyz2, F32)
    nc = _get_nc()
    in_maps, sq1, sq2 = _prep_inputs(xyz1, xyz2)
    global LAST_RESULTS
    LAST_RESULTS = run_bass_kernel_spmd(nc, in_maps, list(range(NCORES)))
    res = LAST_RESULTS.results

    dist1 = np.empty((B, N), F32)
    dist2 = np.empty((B, M), F32)
    idx1 = np.empty((B, N), np.int32)
    idx2 = np.empty((B, M), np.int32)
    PW = 8 * NT + NT  # output cols per problem

    def unpack(outv, p):
        ob = p * PW
        gmax = outv[:, ob + 8 * np.arange(NT)].T.reshape(SLAB)
        gidx = outv[:, ob + 8 * NT:ob + PW].T.reshape(SLAB)
        return gmax, gidx

    for c in range(NCORES):
        outv = np.asarray(res[c]["outv"], F32)  # [TQ, NPROB*PW]
        sl = slice(c * SLAB, (c + 1) * SLAB)
        for b in range(B):
            gmax_f, gidx_f = unpack(outv, b)
            dist1[b, sl] = (sq1[b, sl] - gmax_f).astype(F32)
            idx1[b, sl] = _fixup(gidx_f, dist1[b, sl], xyz1[b, sl],
                                 xyz2[b], -sq2[b])
            gmax_r, gidx_r = unpack(outv, 2 + b)
            dist2[b, sl] = (sq2[b, sl] - gmax_r).astype(F32)
            idx2[b, sl] = _fixup(gidx_r, dist2[b, sl], xyz2[b, sl],
                                 xyz1[b], -sq1[b])
    return dist1, dist2, idx1, idx2

